# revision 4
# baseline (speedup 1.0000x reference)
"""Trainium2 Bass kernel for nn_Attention_12000138625343.

Full multi-head attention layer (B=2, S=2048, E=1024, H=16, hd=64, interleaved
RoPE on q/k, non-causal softmax) run tensor-parallel over 8 NeuronCores:

  - heads sharded 2-per-core (w1 columns / qkv projection sharded),
  - x replicated (passed pre-transposed [E, B*S] so the contraction dim lands
    on SBUF partitions),
  - scores computed transposed [k, q] so softmax exp output feeds the
    attn@v matmul directly as the moving operand,
  - softmax denominator produced by a ones-column appended to v,
  - AllToAll of the per-head attention output o^T (2 MB/rank) converts the
    head sharding into row sharding, then each core runs the w2 projection
    for its own 512 output rows (no AllReduce needed),
  - host concatenates the 8 row blocks.

Matmuls run in float32r (TF32-like, ~1e-4 relative error, full PE rate at
free-dim >= 256 vs 4x slower for plain fp32).
"""

import math

import numpy as np

import concourse.bass as bass
import concourse.mybir as mybir
import concourse.tile as tile
from concourse import bacc
from concourse.bass_utils import run_bass_kernel_spmd

B, S, E, H = 2, 2048, 1024, 16
HD = E // H  # 64
BASE = 10000.0
N_CORES = 8
HPC = H // N_CORES       # heads per core = 2
R = B * S                # 4096 flattened rows
RT = 512                 # rows per r-tile
N_RT = R // RT           # 8 r-tiles
NEC = E // 128           # 8 e-chunks of 128
QT = 512                 # q columns per q-tile
N_QT = S // QT           # 4 q-tiles per batch
KC = 128                 # k rows per k-chunk
N_KC = S // KC           # 16 k-chunks per batch
ROWS_PER_CORE = R // N_CORES  # 512

F32 = mybir.dt.float32
F32R = mybir.dt.float32r
EXPF = mybir.ActivationFunctionType.Exp

_COMPILED = {}


def _build_nc():
    nc = bacc.Bacc("TRN2", target_bir_lowering=False, debug=False,
                   num_devices=N_CORES)

    xT = nc.dram_tensor("xT", [E, R], F32, kind="ExternalInput").ap()
    wqT = nc.dram_tensor("wqT", [E, 128], F32, kind="ExternalInput").ap()
    wkT = nc.dram_tensor("wkT", [E, 128], F32, kind="ExternalInput").ap()
    wvT = nc.dram_tensor("wvT", [E, 128], F32, kind="ExternalInput").ap()
    w2T = nc.dram_tensor("w2T", [E, E], F32, kind="ExternalInput").ap()
    cosT = nc.dram_tensor("cosT", [128, S], F32, kind="ExternalInput").ap()
    sinT = nc.dram_tensor("sinT", [128, S], F32, kind="ExternalInput").ap()
    p2T = nc.dram_tensor("p2T", [128, 128], F32, kind="ExternalInput").ap()
    out = nc.dram_tensor("out", [ROWS_PER_CORE, E], F32,
                         kind="ExternalOutput").ap()

    with tile.TileContext(nc) as tc:
        _emit(tc, nc, xT, wqT, wkT, wvT, w2T, cosT, sinT, p2T, out)
    nc.compile()
    return nc


def _emit(tc, nc, xT, wqT, wkT, wvT, w2T, cosT, sinT, p2T, out):
    import contextlib
    ctx = contextlib.ExitStack()
    consts = ctx.enter_context(tc.tile_pool(name="consts", bufs=1))
    xtp = ctx.enter_context(tc.tile_pool(name="xtp", bufs=2))
    qkp = ctx.enter_context(tc.tile_pool(name="qkp", bufs=1))
    rawp = ctx.enter_context(tc.tile_pool(name="rawp", bufs=1))
    tmpp = ctx.enter_context(tc.tile_pool(name="tmpp", bufs=2))
    vp = ctx.enter_context(tc.tile_pool(name="vp", bufs=1))
    pp = ctx.enter_context(tc.tile_pool(name="pp", bufs=18))
    smallp = ctx.enter_context(tc.tile_pool(name="smallp", bufs=2))
    outp = ctx.enter_context(tc.tile_pool(name="outp", bufs=2))
    dramp = ctx.enter_context(tc.tile_pool(name="dramp", bufs=1, space="DRAM"))
    ps1 = ctx.enter_context(tc.tile_pool(name="ps1", bufs=1, space="PSUM"))
    ps2 = ctx.enter_context(tc.tile_pool(name="ps2", bufs=2, space="PSUM"))

    # ---- constants into SBUF (gpsimd DMA casts f32 -> f32r where needed) ----
    wq_sb, wk_sb, wv_sb, w2_sb = [], [], [], []
    for ec in range(NEC):
        wq_t = consts.tile([128, 128], F32R, tag=f"wq{ec}", name=f"wq{ec}")
        nc.gpsimd.dma_start(out=wq_t[:], in_=wqT[ec * 128:(ec + 1) * 128, :])
        wq_sb.append(wq_t)
        wk_t = consts.tile([128, 128], F32R, tag=f"wk{ec}", name=f"wk{ec}")
        nc.gpsimd.dma_start(out=wk_t[:], in_=wkT[ec * 128:(ec + 1) * 128, :])
        wk_sb.append(wk_t)
        wv_t = consts.tile([128, 128], F32R, tag=f"wv{ec}", name=f"wv{ec}")
        nc.gpsimd.dma_start(out=wv_t[:], in_=wvT[ec * 128:(ec + 1) * 128, :])
        wv_sb.append(wv_t)
        w2_t = consts.tile([128, E], F32R, tag=f"w2{ec}", name=f"w2{ec}")
        nc.gpsimd.dma_start(out=w2_t[:], in_=w2T[ec * 128:(ec + 1) * 128, :])
        w2_sb.append(w2_t)

    cos_sb = consts.tile([128, S], F32, tag="cos", name="cos_sb")
    nc.sync.dma_start(out=cos_sb[:], in_=cosT[:, :])
    sin_sb = consts.tile([128, S], F32, tag="sin", name="sin_sb")
    nc.sync.dma_start(out=sin_sb[:], in_=sinT[:, :])
    p2_sb = consts.tile([128, 128], F32R, tag="p2", name="p2_sb")
    nc.gpsimd.dma_start(out=p2_sb[:], in_=p2T[:, :])
    ones_f32 = consts.tile([128, 64], F32, tag="ones32", name="ones_f32")
    nc.vector.memset(ones_f32[:], 1.0)
    ones_sb = consts.tile([1, 64], F32R, tag="ones", name="ones_sb")
    nc.vector.tensor_copy(ones_sb[:], ones_f32[0:1, 0:64])

    # A2A buffers in DRAM: [8 chunks, 128 e-rows (2 heads), 512 rows]
    send_d = dramp.tile([N_CORES, 128, RT], F32, name="send_d")
    recv_d = dramp.tile([N_CORES, 128, RT], F32, name="recv_d")

    # per (b): qT/kT [128 (2 heads x 64 hd), S] f32r, post-RoPE
    qT_sb = {}
    kT_sb = {}
    # per (b, h, kc): v tiles [128 k, 65] (col 64 = ones)
    v_sb = {}

    def emit_qkv_rtile(rt):
        """Projection + RoPE for r-tile rt (rows rt*512 .. +512, batch rt//4)."""
        b, st = rt // N_QT, (rt % N_QT) * RT  # batch, s-offset in batch
        xt = []
        for ec in range(NEC):
            t = xtp.tile([128, RT], F32R, tag=f"xt{ec}", name=f"xt{ec}_{rt}")
            nc.gpsimd.dma_start(
                out=t[:], in_=xT[ec * 128:(ec + 1) * 128, rt * RT:(rt + 1) * RT])
            xt.append(t)

        if b not in qT_sb:
            qT_sb[b] = qkp.tile([128, S], F32R, tag=f"qT{b}", name=f"qT{b}")
            kT_sb[b] = qkp.tile([128, S], F32R, tag=f"kT{b}", name=f"kT{b}")

        for kind, w_sb, dst in (("q", wq_sb, qT_sb[b]), ("k", wk_sb, kT_sb[b])):
            acc = ps1.tile([128, RT], F32, tag=f"{kind}acc", name=f"{kind}acc{rt}")
            for ec in range(NEC):
                nc.tensor.matmul(acc[:], w_sb[ec][:], xt[ec][:],
                                 start=(ec == 0), stop=(ec == NEC - 1))
            raw = rawp.tile([128, RT], F32R, tag=f"{kind}raw", name=f"{kind}raw{rt}")
            nc.vector.tensor_copy(raw[:], acc[:])
            rot = ps1.tile([128, RT], F32, tag="rot", name=f"{kind}rot{rt}")
            nc.tensor.matmul(rot[:], p2_sb[:], raw[:], start=True, stop=True)
            # final = raw*cos + rot*sin
            t1 = tmpp.tile([128, RT], F32, tag="ropet", name=f"{kind}t1_{rt}")
            nc.vector.tensor_mul(t1[:], raw[:].bitcast(F32),
                                 cos_sb[:, st:st + RT])
            t2 = tmpp.tile([128, RT], F32, tag="ropet", name=f"{kind}t2_{rt}")
            nc.vector.tensor_mul(t2[:], rot[:], sin_sb[:, st:st + RT])
            nc.vector.tensor_add(dst[:, st:st + RT], t1[:], t2[:])

        # v: out [r 128, f 128] accumulated over e-chunks; split into per-head
        # [128, 65] tiles with a ones column for the softmax denominator.
        for sub in range(4):
            vacc = ps1.tile([128, 128], F32, tag="vacc", name=f"vacc{rt}_{sub}")
            for ec in range(NEC):
                nc.tensor.matmul(vacc[:],
                                 xt[ec][:, sub * 128:(sub + 1) * 128],
                                 wv_sb[ec][:],
                                 start=(ec == 0), stop=(ec == NEC - 1))
            kc = (rt % N_QT) * 4 + sub  # k-chunk index within batch
            for h in range(HPC):
                vt = vp.tile([128, 65], F32R, tag=f"v{b}{h}{kc}",
                             name=f"v{b}{h}{kc}")
                nc.vector.tensor_copy(vt[:, 0:64],
                                      vacc[:, h * 64:(h + 1) * 64])
                nc.vector.tensor_copy(vt[:, 64:65], ones_f32[:, 0:1])
                v_sb[(b, h, kc)] = vt

    def emit_scores(b, h, qt):
        """scores^T + exp for (batch, head, q-tile) -> list of 16 p tiles."""
        hof = h * 64
        ptiles = []
        for kc in range(N_KC):
            sps = ps2.tile([128, QT], F32, tag="sps", name=f"s{b}{h}{qt}_{kc}")
            nc.tensor.matmul(
                sps[:],
                kT_sb[b][hof:hof + 64, kc * KC:(kc + 1) * KC],
                qT_sb[b][hof:hof + 64, qt * QT:(qt + 1) * QT],
                start=True, stop=True)
            pt = pp.tile([128, QT], F32R, tag="p", name=f"p{b}{h}{qt}_{kc}")
            nc.scalar.activation(pt[:], sps[:], EXPF, scale=1.0 / math.sqrt(HD))
            ptiles.append(pt)
        return ptiles

    def emit_attnv(b, h, qt, ptiles):
        """attn @ v (+denominator), divide, stage into the A2A send buffer."""
        av = ps1.tile([65, QT], F32, tag="av", name=f"av{b}{h}{qt}")
        for kc in range(N_KC):
            nc.tensor.matmul(av[:], v_sb[(b, h, kc)][:], ptiles[kc][:],
                             start=(kc == 0), stop=(kc == N_KC - 1))
        rcp = smallp.tile([1, QT], F32R, tag="rcp", name=f"rcp{b}{h}{qt}")
        with nc.allow_low_precision(reason="f32r reciprocal, ~1e-4 rel is fine"):
            nc.vector.reciprocal(rcp[:], av[64:65, :])
        bcp = ps1.tile([64, QT], F32, tag="bcp", name=f"bcp{b}{h}{qt}")
        nc.tensor.matmul(bcp[:], ones_sb[:], rcp[:], start=True, stop=True)
        bcs = smallp.tile([64, QT], F32, tag="bcs", name=f"bcs{b}{h}{qt}")
        nc.vector.tensor_copy(bcs[:], bcp[:])
        odiv = smallp.tile([64, QT], F32, tag="odiv", name=f"odiv{b}{h}{qt}")
        nc.vector.tensor_mul(odiv[:], av[0:64, :], bcs[:])
        j = b * N_QT + qt  # destination core
        nc.sync.dma_start(out=send_d[j, h * 64:(h + 1) * 64, :], in_=odiv[:])

    # ---------------- emission ----------------
    for b in range(B):
        for rtl in range(N_QT):
            emit_qkv_rtile(b * N_QT + rtl)
        for h in range(HPC):
            prev = None
            for qt in range(N_QT):
                cur = emit_scores(b, h, qt)
                if prev is not None:
                    emit_attnv(b, h, qt - 1, prev)
                prev = cur
            emit_attnv(b, h, N_QT - 1, prev)

    nc.gpsimd.collective_compute(
        "AllToAll", mybir.AluOpType.bypass,
        replica_groups=[list(range(N_CORES))],
        ins=[send_d.opt()], outs=[recv_d.opt()])

    # ---------------- output projection for my 512 rows ----------------
    recv_sb = []
    for ec in range(NEC):
        t = xtp.tile([128, RT], F32R, tag=f"xt{ec}", name=f"recv{ec}")
        nc.gpsimd.dma_start(out=t[:], in_=recv_d[ec, :, :])
        recv_sb.append(t)
    for rblk in range(4):
        for ft in range(2):
            ops = ps2.tile([128, 512], F32, tag="sps", name=f"ops{rblk}_{ft}")
            for ec in range(NEC):
                nc.tensor.matmul(
                    ops[:],
                    recv_sb[ec][:, rblk * 128:(rblk + 1) * 128],
                    w2_sb[ec][:, ft * 512:(ft + 1) * 512],
                    start=(ec == 0), stop=(ec == NEC - 1))
            ot = outp.tile([128, 512], F32, tag="ot", name=f"ot{rblk}_{ft}")
            nc.vector.tensor_copy(ot[:], ops[:])
            nc.sync.dma_start(
                out=out[rblk * 128:(rblk + 1) * 128, ft * 512:(ft + 1) * 512],
                in_=ot[:])
    ctx.close()


def _host_prep(x, w1, w2):
    x = np.ascontiguousarray(np.asarray(x, dtype=np.float32))
    w1 = np.ascontiguousarray(np.asarray(w1, dtype=np.float32))
    w2 = np.ascontiguousarray(np.asarray(w2, dtype=np.float32))

    xT = np.ascontiguousarray(x.reshape(R, E).T)          # [E, R]
    w2T = np.ascontiguousarray(w2.T)                      # [E, E]

    # RoPE tables in [hd, s] layout, duplicated for the 2 heads per core.
    theta = 1.0 / (BASE ** (np.arange(0, HD, 2, dtype=np.float32) / HD))  # [32]
    enc = np.arange(S, dtype=np.float32)[:, None] * theta[None, :]        # [s,32]
    enc = np.repeat(enc, 2, axis=-1)                                      # [s,64]
    cos1 = np.cos(enc).T.astype(np.float32)               # [64, S]
    sin1 = np.sin(enc).T.astype(np.float32)
    cosT = np.ascontiguousarray(np.concatenate([cos1, cos1], axis=0))  # [128,S]
    sinT = np.ascontiguousarray(np.concatenate([sin1, sin1], axis=0))

    # rotate-half as a matmul: rot = M @ v with M[2i,2i+1]=-1, M[2i+1,2i]=1.
    m64 = np.zeros((HD, HD), dtype=np.float32)
    for i in range(HD // 2):
        m64[2 * i, 2 * i + 1] = -1.0
        m64[2 * i + 1, 2 * i] = 1.0
    m128 = np.zeros((128, 128), dtype=np.float32)
    m128[:64, :64] = m64
    m128[64:, 64:] = m64
    p2T = np.ascontiguousarray(m128.T)

    in_maps = []
    for c in range(N_CORES):
        hA, hB = HPC * c, HPC * c + 1
        def rows(base):
            return np.concatenate(
                [w1[base + hA * HD: base + (hA + 1) * HD, :],
                 w1[base + hB * HD: base + (hB + 1) * HD, :]], axis=0)
        in_maps.append({
            "xT": xT,
            "wqT": np.ascontiguousarray(rows(0).T),
            "wkT": np.ascontiguousarray(rows(E).T),
            "wvT": np.ascontiguousarray(rows(2 * E).T),
            "w2T": w2T,
            "cosT": cosT,
            "sinT": sinT,
            "p2T": p2T,
        })
    return in_maps


def kernel(x, w1, w2, _trace=False):
    if "nc" not in _COMPILED:
        _COMPILED["nc"] = _build_nc()
    nc = _COMPILED["nc"]
    in_maps = _host_prep(x, w1, w2)
    res = run_bass_kernel_spmd(nc, in_maps, core_ids=list(range(N_CORES)),
                               trace=_trace)
    _COMPILED["last_result"] = res
    blocks = [res.results[c]["out"] for c in range(N_CORES)]
    full = np.concatenate(blocks, axis=0).reshape(B, S, E)
    return full.astype(np.float32)


# revision 5
# speedup vs baseline: 1.2594x; 1.2594x over previous
"""Trainium2 Bass kernel for nn_Attention_12000138625343.

Full multi-head attention layer (B=2, S=2048, E=1024, H=16, hd=64, interleaved
RoPE on q/k, non-causal softmax) run tensor-parallel over 8 NeuronCores:

  - heads sharded 2-per-core (w1 columns / qkv projection sharded),
  - x replicated, passed pre-transposed [E, B*S] so the contraction dim lands
    on SBUF partitions,
  - scores computed transposed [k, q]; the two heads' K=64 score matmuls are
    packed into disjoint PE row-groups (concurrent), one exp instruction
    covers both heads' [128, 1024] PSUM block,
  - attn@v accumulates rolling per k-chunk (p tiles freed immediately),
    with a ones-column appended to v producing the softmax denominator,
  - two AllToAlls (one per batch) of the per-head attention output o^T
    convert head sharding into row sharding; the batch-0 A2A and its half of
    the w2 projection hide under batch-1 compute,
  - each core owns 256 rows of each batch; host reassembles.

Matmuls run in float32r (TF32-like, ~1e-4 relative error, full PE rate at
free-dim >= 256).
"""

import math

import numpy as np

import concourse.bass as bass
import concourse.mybir as mybir
import concourse.tile as tile
from concourse import bacc
from concourse.bass_utils import run_bass_kernel_spmd

B, S, E, H = 2, 2048, 1024, 16
HD = E // H  # 64
BASE = 10000.0
N_CORES = 8
HPC = H // N_CORES       # heads per core = 2
R = B * S                # 4096 flattened rows
RT = 512                 # rows per r-tile
NEC = E // 128           # 8 e-chunks of 128
QT = 512                 # q columns per q-tile
N_QT = S // QT           # 4 q-tiles per batch
KC = 128                 # k rows per k-chunk
N_KC = S // KC           # 16 k-chunks per batch
RPB = S // N_CORES       # rows per core per batch = 256

F32 = mybir.dt.float32
F32R = mybir.dt.float32r
EXPF = mybir.ActivationFunctionType.Exp

_COMPILED = {}


def _build_nc():
    nc = bacc.Bacc("TRN2", target_bir_lowering=False, debug=False,
                   num_devices=N_CORES)

    xT = nc.dram_tensor("xT", [E, R], F32, kind="ExternalInput").ap()
    wqT = nc.dram_tensor("wqT", [E, 128], F32, kind="ExternalInput").ap()
    wkT = nc.dram_tensor("wkT", [E, 128], F32, kind="ExternalInput").ap()
    wvT = nc.dram_tensor("wvT", [E, 128], F32, kind="ExternalInput").ap()
    w2T = nc.dram_tensor("w2T", [E, E], F32, kind="ExternalInput").ap()
    cosT = nc.dram_tensor("cosT", [128, S], F32, kind="ExternalInput").ap()
    sinT = nc.dram_tensor("sinT", [128, S], F32, kind="ExternalInput").ap()
    p2T = nc.dram_tensor("p2T", [128, 128], F32, kind="ExternalInput").ap()
    out = nc.dram_tensor("out", [2 * RPB, E], F32, kind="ExternalOutput").ap()

    with tile.TileContext(nc) as tc:
        _emit(tc, nc, xT, wqT, wkT, wvT, w2T, cosT, sinT, p2T, out)
    nc.compile()
    return nc


def _emit(tc, nc, xT, wqT, wkT, wvT, w2T, cosT, sinT, p2T, out):
    import contextlib
    ctx = contextlib.ExitStack()
    consts = ctx.enter_context(tc.tile_pool(name="consts", bufs=1))
    xtp = ctx.enter_context(tc.tile_pool(name="xtp", bufs=2))
    qkp = ctx.enter_context(tc.tile_pool(name="qkp", bufs=1))
    rawp = ctx.enter_context(tc.tile_pool(name="rawp", bufs=1))
    tmpp = ctx.enter_context(tc.tile_pool(name="tmpp", bufs=2))
    vp = ctx.enter_context(tc.tile_pool(name="vp", bufs=1))
    pp = ctx.enter_context(tc.tile_pool(name="pp", bufs=6))
    smallp = ctx.enter_context(tc.tile_pool(name="smallp", bufs=2))
    dramp = ctx.enter_context(tc.tile_pool(name="dramp", bufs=1, space="DRAM"))
    # PSUM budget (8 banks): qkv-shared 2 x 1 + sps 2 x 2 + av 2 x 1 = 8
    ps_qkv = ctx.enter_context(tc.tile_pool(name="ps_qkv", bufs=2, space="PSUM"))
    ps_sps = ctx.enter_context(tc.tile_pool(name="ps_sps", bufs=2, space="PSUM"))
    ps_av = ctx.enter_context(tc.tile_pool(name="ps_av", bufs=2, space="PSUM"))

    # ---- qkv weights + small consts first (everything else is emitted late) ----
    wq_sb, wk_sb, wv_sb = [], [], []
    for ec in range(NEC):
        wq_t = consts.tile([128, 128], F32R, tag=f"wq{ec}", name=f"wq{ec}")
        nc.gpsimd.dma_start(out=wq_t[:], in_=wqT[ec * 128:(ec + 1) * 128, :])
        wq_sb.append(wq_t)
        wk_t = consts.tile([128, 128], F32R, tag=f"wk{ec}", name=f"wk{ec}")
        nc.gpsimd.dma_start(out=wk_t[:], in_=wkT[ec * 128:(ec + 1) * 128, :])
        wk_sb.append(wk_t)
        wv_t = consts.tile([128, 128], F32R, tag=f"wv{ec}", name=f"wv{ec}")
        nc.gpsimd.dma_start(out=wv_t[:], in_=wvT[ec * 128:(ec + 1) * 128, :])
        wv_sb.append(wv_t)

    cos_sb = consts.tile([128, S], F32, tag="cos", name="cos_sb")
    nc.sync.dma_start(out=cos_sb[:], in_=cosT[:, :])
    sin_sb = consts.tile([128, S], F32, tag="sin", name="sin_sb")
    nc.sync.dma_start(out=sin_sb[:], in_=sinT[:, :])
    p2_sb = consts.tile([128, 128], F32R, tag="p2", name="p2_sb")
    nc.gpsimd.dma_start(out=p2_sb[:], in_=p2T[:, :])
    ones_f32 = consts.tile([128, 64], F32, tag="ones32", name="ones_f32")
    nc.vector.memset(ones_f32[:], 1.0)
    ones_sb = consts.tile([1, 64], F32R, tag="ones", name="ones_sb")
    nc.vector.tensor_copy(ones_sb[:], ones_f32[0:1, 0:64])

    # A2A buffers, one pair per batch: [8 chunks, 128 e-rows, 256 rows]
    send_d = [dramp.tile([N_CORES, 128, RPB], F32, name=f"send{b}")
              for b in range(B)]
    recv_d = [dramp.tile([N_CORES, 128, RPB], F32, name=f"recv{b}")
              for b in range(B)]

    qT_sb, kT_sb, v_sb, w2_sb = {}, {}, {}, []

    def emit_qkv_rtile(rt):
        """Projection + RoPE for r-tile rt (rows rt*512 .. +512, batch rt//4)."""
        b, st = rt // N_QT, (rt % N_QT) * RT
        xt = []
        for ec in range(NEC):
            t = xtp.tile([128, RT], F32R, tag=f"xt{ec}", name=f"xt{ec}_{rt}")
            nc.gpsimd.dma_start(
                out=t[:], in_=xT[ec * 128:(ec + 1) * 128, rt * RT:(rt + 1) * RT])
            xt.append(t)

        if b not in qT_sb:
            qT_sb[b] = qkp.tile([128, S], F32R, tag=f"qT{b}", name=f"qT{b}")
            kT_sb[b] = qkp.tile([128, S], F32R, tag=f"kT{b}", name=f"kT{b}")

        for kind, w_sb, dst in (("q", wq_sb, qT_sb[b]), ("k", wk_sb, kT_sb[b])):
            acc = ps_qkv.tile([128, RT], F32, tag="qkv", name=f"{kind}acc{rt}")
            for ec in range(NEC):
                nc.tensor.matmul(acc[:], w_sb[ec][:], xt[ec][:],
                                 start=(ec == 0), stop=(ec == NEC - 1))
            raw = rawp.tile([128, RT], F32R, tag=f"{kind}raw", name=f"{kind}raw{rt}")
            nc.vector.tensor_copy(raw[:], acc[:])
            rot = ps_qkv.tile([128, RT], F32, tag="qkv", name=f"{kind}rot{rt}")
            nc.tensor.matmul(rot[:], p2_sb[:], raw[:], start=True, stop=True)
            t1 = tmpp.tile([128, RT], F32, tag="ropet", name=f"{kind}t1_{rt}")
            nc.vector.tensor_mul(t1[:], raw[:].bitcast(F32),
                                 cos_sb[:, st:st + RT])
            t2 = tmpp.tile([128, RT], F32, tag="ropet", name=f"{kind}t2_{rt}")
            nc.vector.tensor_mul(t2[:], rot[:], sin_sb[:, st:st + RT])
            nc.vector.tensor_add(dst[:, st:st + RT], t1[:], t2[:])

        for sub in range(4):
            vacc = ps_qkv.tile([128, 128], F32, tag="qkv", name=f"vacc{rt}_{sub}")
            for ec in range(NEC):
                nc.tensor.matmul(vacc[:],
                                 xt[ec][:, sub * 128:(sub + 1) * 128],
                                 wv_sb[ec][:],
                                 start=(ec == 0), stop=(ec == NEC - 1))
            kc = (rt % N_QT) * 4 + sub
            for h in range(HPC):
                vt = vp.tile([128, 65], F32R, tag=f"v{b}{h}{kc}",
                             name=f"v{b}{h}{kc}")
                nc.vector.tensor_copy(vt[:, 0:64],
                                      vacc[:, h * 64:(h + 1) * 64])
                nc.vector.tensor_copy(vt[:, 64:65], ones_f32[:, 0:1])
                v_sb[(b, h, kc)] = vt

    def emit_attention_qt(b, qt):
        """Both heads for one q-tile: packed scores, combined exp, rolling
        attn@v, divide, stage into the A2A send buffer."""
        scale = 1.0 / math.sqrt(HD)
        avs = [ps_av.tile([65, QT], F32, tag="av", name=f"av{b}{h}{qt}")
               for h in range(HPC)]
        for kc in range(N_KC):
            sps = ps_sps.tile([128, 2 * QT], F32, tag="sps",
                              name=f"s{b}{qt}_{kc}")
            for h in range(HPC):
                hof = h * 64
                nc.tensor.matmul(
                    sps[:, h * QT:(h + 1) * QT],
                    kT_sb[b][hof:hof + 64, kc * KC:(kc + 1) * KC],
                    qT_sb[b][hof:hof + 64, qt * QT:(qt + 1) * QT],
                    start=True, stop=True)
            pt = pp.tile([128, 2 * QT], F32R, tag="p", name=f"p{b}{qt}_{kc}")
            nc.scalar.activation(pt[:], sps[:], EXPF, scale=scale)
            for h in range(HPC):
                nc.tensor.matmul(avs[h][:], v_sb[(b, h, kc)][:],
                                 pt[:, h * QT:(h + 1) * QT],
                                 start=(kc == 0), stop=(kc == N_KC - 1))
        for h in range(HPC):
            av = avs[h]
            rcp = smallp.tile([1, QT], F32R, tag="rcp", name=f"rcp{b}{h}{qt}")
            with nc.allow_low_precision(reason="f32r reciprocal ~1e-4"):
                nc.vector.reciprocal(rcp[:], av[64:65, :])
            bcp = ps_qkv.tile([64, QT], F32, tag="qkv", name=f"bcp{b}{h}{qt}")
            nc.tensor.matmul(bcp[:], ones_sb[:], rcp[:], start=True, stop=True)
            bcs = smallp.tile([64, QT], F32, tag="bcs", name=f"bcs{b}{h}{qt}")
            nc.vector.tensor_copy(bcs[:], bcp[:])
            odiv = smallp.tile([64, QT], F32, tag="odiv", name=f"odiv{b}{h}{qt}")
            nc.vector.tensor_mul(odiv[:], av[0:64, :], bcs[:])
            # q-tile qt covers destination cores 2*qt (first 256 cols) and
            # 2*qt+1 (last 256 cols)
            nc.sync.dma_start(out=send_d[b][2 * qt, h * 64:(h + 1) * 64, :],
                              in_=odiv[:, 0:RPB])
            nc.sync.dma_start(out=send_d[b][2 * qt + 1, h * 64:(h + 1) * 64, :],
                              in_=odiv[:, RPB:2 * RPB])

    def emit_a2a_and_proj(b):
        nc.gpsimd.collective_compute(
            "AllToAll", mybir.AluOpType.bypass,
            replica_groups=[list(range(N_CORES))],
            ins=[send_d[b].opt()], outs=[recv_d[b].opt()])
        recv_sb = []
        for ec in range(NEC):
            t = xtp.tile([128, RPB], F32R, tag=f"xt{ec}", name=f"recv{b}_{ec}")
            nc.gpsimd.dma_start(out=t[:], in_=recv_d[b][ec, :, :])
            recv_sb.append(t)
        for rblk in range(RPB // 128):
            for ft in range(2):
                ops = ps_sps.tile([128, 512], F32, tag="sps",
                                  name=f"ops{b}_{rblk}_{ft}")
                for ec in range(NEC):
                    nc.tensor.matmul(
                        ops[:],
                        recv_sb[ec][:, rblk * 128:(rblk + 1) * 128],
                        w2_sb[ec][:, ft * 512:(ft + 1) * 512],
                        start=(ec == 0), stop=(ec == NEC - 1))
                ot = tmpp.tile([128, 512], F32, tag="ropet",
                               name=f"ot{b}_{rblk}_{ft}")
                nc.vector.tensor_copy(ot[:], ops[:])
                nc.sync.dma_start(
                    out=out[b * RPB + rblk * 128:b * RPB + (rblk + 1) * 128,
                            ft * 512:(ft + 1) * 512],
                    in_=ot[:])

    # ---------------- emission ----------------
    for rt in range(N_QT):             # batch-0 projection
        emit_qkv_rtile(rt)
    for qt in range(N_QT):             # batch-0 attention, b1 qkv interleaved
        emit_attention_qt(0, qt)
        emit_qkv_rtile(N_QT + qt)
    # w2 chunks (needed from the first out-projection onward)
    for ec in range(NEC):
        w2_t = consts.tile([128, E], F32R, tag=f"w2{ec}", name=f"w2{ec}")
        nc.gpsimd.dma_start(out=w2_t[:], in_=w2T[ec * 128:(ec + 1) * 128, :])
        w2_sb.append(w2_t)
    emit_a2a_and_proj(0)               # hides under batch-1 attention
    for qt in range(N_QT):
        emit_attention_qt(1, qt)
    emit_a2a_and_proj(1)
    ctx.close()


def _host_prep(x, w1, w2):
    x = np.ascontiguousarray(np.asarray(x, dtype=np.float32))
    w1 = np.ascontiguousarray(np.asarray(w1, dtype=np.float32))
    w2 = np.ascontiguousarray(np.asarray(w2, dtype=np.float32))

    xT = np.ascontiguousarray(x.reshape(R, E).T)          # [E, R]
    w2T = np.ascontiguousarray(w2.T)                      # [E, E]

    theta = 1.0 / (BASE ** (np.arange(0, HD, 2, dtype=np.float32) / HD))
    enc = np.arange(S, dtype=np.float32)[:, None] * theta[None, :]
    enc = np.repeat(enc, 2, axis=-1)                      # [s, 64]
    cos1 = np.cos(enc).T.astype(np.float32)               # [64, S]
    sin1 = np.sin(enc).T.astype(np.float32)
    cosT = np.ascontiguousarray(np.concatenate([cos1, cos1], axis=0))
    sinT = np.ascontiguousarray(np.concatenate([sin1, sin1], axis=0))

    m64 = np.zeros((HD, HD), dtype=np.float32)
    for i in range(HD // 2):
        m64[2 * i, 2 * i + 1] = -1.0
        m64[2 * i + 1, 2 * i] = 1.0
    m128 = np.zeros((128, 128), dtype=np.float32)
    m128[:64, :64] = m64
    m128[64:, 64:] = m64
    p2T = np.ascontiguousarray(m128.T)

    in_maps = []
    for c in range(N_CORES):
        hA, hB = HPC * c, HPC * c + 1
        def rows(base):
            return np.concatenate(
                [w1[base + hA * HD: base + (hA + 1) * HD, :],
                 w1[base + hB * HD: base + (hB + 1) * HD, :]], axis=0)
        in_maps.append({
            "xT": xT,
            "wqT": np.ascontiguousarray(rows(0).T),
            "wkT": np.ascontiguousarray(rows(E).T),
            "wvT": np.ascontiguousarray(rows(2 * E).T),
            "w2T": w2T,
            "cosT": cosT,
            "sinT": sinT,
            "p2T": p2T,
        })
    return in_maps


def kernel(x, w1, w2, _trace=False):
    if "nc" not in _COMPILED:
        _COMPILED["nc"] = _build_nc()
    nc = _COMPILED["nc"]
    in_maps = _host_prep(x, w1, w2)
    res = run_bass_kernel_spmd(nc, in_maps, core_ids=list(range(N_CORES)),
                               trace=_trace)
    _COMPILED["last_result"] = res
    # core c returns [512, E]: rows 0..255 = batch0 s in [256c, 256c+256),
    # rows 256..511 = batch1 same s range.
    full = np.empty((B, S, E), dtype=np.float32)
    for c in range(N_CORES):
        blk = res.results[c]["out"]
        full[0, c * RPB:(c + 1) * RPB] = blk[0:RPB]
        full[1, c * RPB:(c + 1) * RPB] = blk[RPB:2 * RPB]
    return full


# revision 11
# speedup vs baseline: 1.4143x; 1.1230x over previous
"""Trainium2 Bass kernel for nn_Attention_12000138625343.

Full multi-head attention layer (B=2, S=2048, E=1024, H=16, hd=64, interleaved
RoPE on q/k, non-causal softmax) run tensor-parallel over 8 NeuronCores:

  - heads sharded 2-per-core (w1 columns / qkv projection sharded),
  - x replicated, passed pre-transposed [E, B*S] so the contraction dim lands
    on SBUF partitions,
  - scores computed transposed [k, q]; the two heads' K=64 score matmuls are
    packed into disjoint PE row-groups (concurrent), one exp instruction
    covers both heads' [128, 1024] PSUM block,
  - attn@v accumulates rolling per k-chunk (p tiles freed immediately),
    with a ones-column appended to v producing the softmax denominator,
  - two AllToAlls (one per batch) of the per-head attention output o^T
    convert head sharding into row sharding; the batch-0 A2A and its half of
    the w2 projection hide under batch-1 compute,
  - each core owns 256 rows of each batch; host reassembles.

Matmuls run in float32r (TF32-like, ~1e-4 relative error, full PE rate at
free-dim >= 256).
"""

import math

import numpy as np

import concourse.bass as bass
import concourse.mybir as mybir
import concourse.tile as tile
from concourse import bacc
from concourse.bass_utils import run_bass_kernel_spmd

B, S, E, H = 2, 2048, 1024, 16
HD = E // H  # 64
BASE = 10000.0
N_CORES = 8
HPC = H // N_CORES       # heads per core = 2
R = B * S                # 4096 flattened rows
RT = 512                 # rows per r-tile
NEC = E // 128           # 8 e-chunks of 128
QT = 512                 # q columns per q-tile
N_QT = S // QT           # 4 q-tiles per batch
KC = 128                 # k rows per k-chunk
N_KC = S // KC           # 16 k-chunks per batch
RPB = S // N_CORES       # rows per core per batch = 256

F32 = mybir.dt.float32
F32R = mybir.dt.float32r
EXPF = mybir.ActivationFunctionType.Exp

_COMPILED = {}


def _build_nc():
    nc = bacc.Bacc("TRN2", target_bir_lowering=False, debug=False,
                   num_devices=N_CORES)

    xT = nc.dram_tensor("xT", [E, R], F32, kind="ExternalInput").ap()
    wqT = nc.dram_tensor("wqT", [E, 128], F32, kind="ExternalInput").ap()
    wkT = nc.dram_tensor("wkT", [E, 128], F32, kind="ExternalInput").ap()
    wvT = nc.dram_tensor("wvT", [E, 128], F32, kind="ExternalInput").ap()
    w2T = nc.dram_tensor("w2T", [E, E], F32, kind="ExternalInput").ap()
    cosT = nc.dram_tensor("cosT", [128, S], F32, kind="ExternalInput").ap()
    sinT = nc.dram_tensor("sinT", [128, S], F32, kind="ExternalInput").ap()
    p2T = nc.dram_tensor("p2T", [128, 128], F32, kind="ExternalInput").ap()
    out = nc.dram_tensor("out", [2 * RPB, E], F32, kind="ExternalOutput").ap()

    with tile.TileContext(nc) as tc:
        _emit(tc, nc, xT, wqT, wkT, wvT, w2T, cosT, sinT, p2T, out)
    nc.compile()
    return nc


def _emit(tc, nc, xT, wqT, wkT, wvT, w2T, cosT, sinT, p2T, out):
    import contextlib
    ctx = contextlib.ExitStack()
    consts = ctx.enter_context(tc.tile_pool(name="consts", bufs=1))
    xtp = ctx.enter_context(tc.tile_pool(name="xtp", bufs=2))
    qkp = ctx.enter_context(tc.tile_pool(name="qkp", bufs=1))
    rawp = ctx.enter_context(tc.tile_pool(name="rawp", bufs=1))
    tmpp = ctx.enter_context(tc.tile_pool(name="tmpp", bufs=2))
    vp = ctx.enter_context(tc.tile_pool(name="vp", bufs=1))
    pp = ctx.enter_context(tc.tile_pool(name="pp", bufs=6))
    smallp = ctx.enter_context(tc.tile_pool(name="smallp", bufs=2))
    dramp = ctx.enter_context(tc.tile_pool(name="dramp", bufs=1, space="DRAM"))
    # PSUM budget (8 banks): qkv-shared 2 x 1 + sps 2 x 2 + av 2 x 1 = 8
    ps_qkv = ctx.enter_context(tc.tile_pool(name="ps_qkv", bufs=2, space="PSUM"))
    ps_sps = ctx.enter_context(tc.tile_pool(name="ps_sps", bufs=2, space="PSUM"))
    ps_av = ctx.enter_context(tc.tile_pool(name="ps_av", bufs=2, space="PSUM"))

    # ---- qkv weights + small consts first (everything else is emitted late) ----
    wq_sb, wk_sb, wv_sb = [], [], []
    for ec in range(NEC):
        wq_t = consts.tile([128, 128], F32R, tag=f"wq{ec}", name=f"wq{ec}")
        nc.gpsimd.dma_start(out=wq_t[:], in_=wqT[ec * 128:(ec + 1) * 128, :])
        wq_sb.append(wq_t)
        wk_t = consts.tile([128, 128], F32R, tag=f"wk{ec}", name=f"wk{ec}")
        nc.gpsimd.dma_start(out=wk_t[:], in_=wkT[ec * 128:(ec + 1) * 128, :])
        wk_sb.append(wk_t)
        wv_t = consts.tile([128, 128], F32R, tag=f"wv{ec}", name=f"wv{ec}")
        nc.gpsimd.dma_start(out=wv_t[:], in_=wvT[ec * 128:(ec + 1) * 128, :])
        wv_sb.append(wv_t)

    cos_sb = consts.tile([128, S], F32, tag="cos", name="cos_sb")
    nc.sync.dma_start(out=cos_sb[:], in_=cosT[:, :])
    sin_sb = consts.tile([128, S], F32, tag="sin", name="sin_sb")
    nc.sync.dma_start(out=sin_sb[:], in_=sinT[:, :])
    p2_sb = consts.tile([128, 128], F32R, tag="p2", name="p2_sb")
    nc.gpsimd.dma_start(out=p2_sb[:], in_=p2T[:, :])
    ones_f32 = consts.tile([128, 64], F32, tag="ones32", name="ones_f32")
    nc.vector.memset(ones_f32[:], 1.0)

    # A2A buffers, one pair per batch: [8 chunks, 128 e-rows, 256 rows]
    send_d = [dramp.tile([N_CORES, 128, RPB], F32, name=f"send{b}")
              for b in range(B)]
    recv_d = [dramp.tile([N_CORES, 128, RPB], F32, name=f"recv{b}")
              for b in range(B)]

    qT_sb, kT_sb, v_sb, w2_sb = {}, {}, {}, []

    def emit_qkv_rtile(rt):
        """Projection + RoPE for r-tile rt (rows rt*512 .. +512, batch rt//4)."""
        b, st = rt // N_QT, (rt % N_QT) * RT
        xt = []
        for ec in range(NEC):
            t = xtp.tile([128, RT], F32R, tag=f"xt{ec}", name=f"xt{ec}_{rt}")
            nc.gpsimd.dma_start(
                out=t[:], in_=xT[ec * 128:(ec + 1) * 128, rt * RT:(rt + 1) * RT])
            xt.append(t)

        if b not in qT_sb:
            qT_sb[b] = qkp.tile([128, S], F32R, tag=f"qT{b}", name=f"qT{b}")
            kT_sb[b] = qkp.tile([128, S], F32R, tag=f"kT{b}", name=f"kT{b}")

        for kind, w_sb, dst in (("q", wq_sb, qT_sb[b]), ("k", wk_sb, kT_sb[b])):
            acc = ps_qkv.tile([128, RT], F32, tag="qkv", name=f"{kind}acc{rt}")
            for ec in range(NEC):
                nc.tensor.matmul(acc[:], w_sb[ec][:], xt[ec][:],
                                 start=(ec == 0), stop=(ec == NEC - 1))
            raw = rawp.tile([128, RT], F32R, tag=f"{kind}raw", name=f"{kind}raw{rt}")
            nc.vector.tensor_copy(raw[:], acc[:])
            rot = ps_qkv.tile([128, RT], F32, tag="qkv", name=f"{kind}rot{rt}")
            nc.tensor.matmul(rot[:], p2_sb[:], raw[:], start=True, stop=True)
            t1 = tmpp.tile([128, RT], F32, tag="ropet", name=f"{kind}t1_{rt}")
            nc.vector.tensor_mul(t1[:], raw[:].bitcast(F32),
                                 cos_sb[:, st:st + RT])
            t2 = tmpp.tile([128, RT], F32, tag="ropet", name=f"{kind}t2_{rt}")
            nc.vector.tensor_mul(t2[:], rot[:], sin_sb[:, st:st + RT])
            nc.vector.tensor_add(dst[:, st:st + RT], t1[:], t2[:])

        for sub in range(4):
            vacc = ps_qkv.tile([128, 128], F32, tag="qkv", name=f"vacc{rt}_{sub}")
            for ec in range(NEC):
                nc.tensor.matmul(vacc[:],
                                 xt[ec][:, sub * 128:(sub + 1) * 128],
                                 wv_sb[ec][:],
                                 start=(ec == 0), stop=(ec == NEC - 1))
            kc = (rt % N_QT) * 4 + sub
            for h in range(HPC):
                vt = vp.tile([128, 65], F32R, tag=f"v{b}{h}{kc}",
                             name=f"v{b}{h}{kc}")
                nc.vector.tensor_copy(vt[:, 0:64],
                                      vacc[:, h * 64:(h + 1) * 64])
                nc.vector.tensor_copy(vt[:, 64:65], ones_f32[:, 0:1])
                v_sb[(b, h, kc)] = vt

    def emit_attention_qt(b, qt):
        """Both heads for one q-tile: packed scores, combined exp, rolling
        attn@v. Returns the two [65, QT] PSUM accumulators."""
        scale = 1.0 / math.sqrt(HD)
        avs = [ps_av.tile([65, QT], F32, tag="av", name=f"av{b}{h}{qt}")
               for h in range(HPC)]
        LAG = 3   # attn@v trails scores by LAG k-chunks so an av-slot stall
                  # never blocks the score->exp pipeline (ACT is the bottleneck)
        pts = {}
        for kc in range(N_KC + LAG):
            if kc < N_KC:
                sps = ps_sps.tile([128, 2 * QT], F32, tag="sps",
                                  name=f"s{b}{qt}_{kc}")
                for h in range(HPC):
                    hof = h * 64
                    nc.tensor.matmul(
                        sps[:, h * QT:(h + 1) * QT],
                        kT_sb[b][hof:hof + 64, kc * KC:(kc + 1) * KC],
                        qT_sb[b][hof:hof + 64, qt * QT:(qt + 1) * QT],
                        start=True, stop=True)
                pt = pp.tile([128, 2 * QT], F32R, tag="p", name=f"p{b}{qt}_{kc}")
                nc.scalar.activation(pt[:], sps[:], EXPF, scale=scale)
                pts[kc] = pt
            if kc >= LAG:
                j = kc - LAG
                for h in range(HPC):
                    nc.tensor.matmul(avs[h][:], v_sb[(b, h, j)][:],
                                     pts[j][:, h * QT:(h + 1) * QT],
                                     start=(j == 0), stop=(j == N_KC - 1))
                del pts[j]
        return avs

    def emit_divide(b, qt, avs):
        """Divide by the softmax denominator (row 64 of av) and stage into
        the A2A send buffer. PE-free: broadcast via a DRAM bounce DMA."""
        for h in range(HPC):
            av = avs[h]
            rcp = smallp.tile([1, QT], F32, tag="rcp", name=f"rcp{b}{h}{qt}")
            nc.vector.reciprocal(rcp[:], av[64:65, :])
            rcp_d = dramp.tile([1, QT], F32, tag="rcpd", bufs=4,
                               name=f"rcpd{b}{h}{qt}")
            nc.sync.dma_start(out=rcp_d[:], in_=rcp[:])
            bcs = smallp.tile([64, QT], F32, tag="bcs", name=f"bcs{b}{h}{qt}")
            bcast = bass.AP(tensor=rcp_d.tensor, offset=rcp_d.offset,
                            ap=[[0, 64]] + list(rcp_d.ap[1:]))
            nc.gpsimd.dma_start(out=bcs[:], in_=bcast)
            odiv = smallp.tile([64, QT], F32, tag="odiv", name=f"odiv{b}{h}{qt}")
            nc.vector.tensor_mul(odiv[:], av[0:64, :], bcs[:])
            # q-tile qt covers destination cores 2*qt (first 256 cols) and
            # 2*qt+1 (last 256 cols)
            nc.sync.dma_start(out=send_d[b][2 * qt, h * 64:(h + 1) * 64, :],
                              in_=odiv[:, 0:RPB])
            nc.sync.dma_start(out=send_d[b][2 * qt + 1, h * 64:(h + 1) * 64, :],
                              in_=odiv[:, RPB:2 * RPB])

    def emit_a2a(b):
        nc.gpsimd.collective_compute(
            "AllToAll", mybir.AluOpType.bypass,
            replica_groups=[list(range(N_CORES))],
            ins=[send_d[b].opt()], outs=[recv_d[b].opt()])

    def emit_proj(b):
        recv_sb = []
        for ec in range(NEC):
            t = xtp.tile([128, RPB], F32R, tag=f"xt{ec}", name=f"recv{b}_{ec}")
            nc.gpsimd.dma_start(out=t[:], in_=recv_d[b][ec, :, :])
            recv_sb.append(t)
        for rblk in range(RPB // 128):
            for ft in range(2):
                ops = ps_sps.tile([128, 512], F32, tag="sps",
                                  name=f"ops{b}_{rblk}_{ft}")
                for ec in range(NEC):
                    nc.tensor.matmul(
                        ops[:],
                        recv_sb[ec][:, rblk * 128:(rblk + 1) * 128],
                        w2_sb[ec][:, ft * 512:(ft + 1) * 512],
                        start=(ec == 0), stop=(ec == NEC - 1))
                ot = tmpp.tile([128, 512], F32, tag="ropet",
                               name=f"ot{b}_{rblk}_{ft}")
                nc.vector.tensor_copy(ot[:], ops[:])
                nc.sync.dma_start(
                    out=out[b * RPB + rblk * 128:b * RPB + (rblk + 1) * 128,
                            ft * 512:(ft + 1) * 512],
                    in_=ot[:])

    # ---------------- emission ----------------
    for rt in range(N_QT):             # batch-0 projection
        emit_qkv_rtile(rt)
    for qt in range(N_QT):             # batch-0 attention, b1 qkv interleaved
        avs = emit_attention_qt(0, qt)
        emit_divide(0, qt, avs)
        emit_qkv_rtile(N_QT + qt)
    # w2 chunks (needed from the first out-projection onward)
    for ec in range(NEC):
        w2_t = consts.tile([128, E], F32R, tag=f"w2{ec}", name=f"w2{ec}")
        nc.gpsimd.dma_start(out=w2_t[:], in_=w2T[ec * 128:(ec + 1) * 128, :])
        w2_sb.append(w2_t)
    emit_a2a(0)                        # in flight under batch-1 attention
    for qt in range(N_QT):
        avs = emit_attention_qt(1, qt)
        emit_divide(1, qt, avs)
        if qt == 2:
            emit_proj(0)               # A2A#0 done by now; fills PE slack
    emit_a2a(1)
    emit_proj(1)
    ctx.close()


def _host_prep(x, w1, w2):
    x = np.ascontiguousarray(np.asarray(x, dtype=np.float32))
    w1 = np.ascontiguousarray(np.asarray(w1, dtype=np.float32))
    w2 = np.ascontiguousarray(np.asarray(w2, dtype=np.float32))

    xT = np.ascontiguousarray(x.reshape(R, E).T)          # [E, R]
    w2T = np.ascontiguousarray(w2.T)                      # [E, E]

    theta = 1.0 / (BASE ** (np.arange(0, HD, 2, dtype=np.float32) / HD))
    enc = np.arange(S, dtype=np.float32)[:, None] * theta[None, :]
    enc = np.repeat(enc, 2, axis=-1)                      # [s, 64]
    cos1 = np.cos(enc).T.astype(np.float32)               # [64, S]
    sin1 = np.sin(enc).T.astype(np.float32)
    cosT = np.ascontiguousarray(np.concatenate([cos1, cos1], axis=0))
    sinT = np.ascontiguousarray(np.concatenate([sin1, sin1], axis=0))

    m64 = np.zeros((HD, HD), dtype=np.float32)
    for i in range(HD // 2):
        m64[2 * i, 2 * i + 1] = -1.0
        m64[2 * i + 1, 2 * i] = 1.0
    m128 = np.zeros((128, 128), dtype=np.float32)
    m128[:64, :64] = m64
    m128[64:, 64:] = m64
    p2T = np.ascontiguousarray(m128.T)

    in_maps = []
    for c in range(N_CORES):
        hA, hB = HPC * c, HPC * c + 1
        def rows(base):
            return np.concatenate(
                [w1[base + hA * HD: base + (hA + 1) * HD, :],
                 w1[base + hB * HD: base + (hB + 1) * HD, :]], axis=0)
        in_maps.append({
            "xT": xT,
            "wqT": np.ascontiguousarray(rows(0).T),
            "wkT": np.ascontiguousarray(rows(E).T),
            "wvT": np.ascontiguousarray(rows(2 * E).T),
            "w2T": w2T,
            "cosT": cosT,
            "sinT": sinT,
            "p2T": p2T,
        })
    return in_maps


def kernel(x, w1, w2, _trace=False):
    if "nc" not in _COMPILED:
        _COMPILED["nc"] = _build_nc()
    nc = _COMPILED["nc"]
    in_maps = _host_prep(x, w1, w2)
    res = run_bass_kernel_spmd(nc, in_maps, core_ids=list(range(N_CORES)),
                               trace=_trace)
    _COMPILED["last_result"] = res
    # core c returns [512, E]: rows 0..255 = batch0 s in [256c, 256c+256),
    # rows 256..511 = batch1 same s range.
    full = np.empty((B, S, E), dtype=np.float32)
    for c in range(N_CORES):
        blk = res.results[c]["out"]
        full[0, c * RPB:(c + 1) * RPB] = blk[0:RPB]
        full[1, c * RPB:(c + 1) * RPB] = blk[RPB:2 * RPB]
    return full


# revision 13
# speedup vs baseline: 1.4495x; 1.0249x over previous
"""Trainium2 Bass kernel for nn_Attention_12000138625343.

Full multi-head attention layer (B=2, S=2048, E=1024, H=16, hd=64, interleaved
RoPE on q/k, non-causal softmax) run tensor-parallel over 8 NeuronCores:

  - heads sharded 2-per-core (w1 columns / qkv projection sharded),
  - x replicated, passed pre-transposed [E, B*S] so the contraction dim lands
    on SBUF partitions,
  - scores computed transposed [k, q]; the two heads' K=64 score matmuls are
    packed into disjoint PE row-groups (concurrent), one exp instruction
    covers both heads' [128, 1024] PSUM block,
  - attn@v accumulates rolling per k-chunk with a ones-column appended to v
    producing the softmax denominator; the divide runs entirely off the
    TensorEngine (DVE reciprocal + DRAM-bounce broadcast DMA + DVE multiply),
  - batch-1 qkv projection / batch-0 output projection matmul chains are
    dribbled into the attention k-chunk loop so the in-order PE stream never
    starves the exp pipeline for long,
  - two AllToAlls (one per batch) of the per-head attention output o^T
    convert head sharding into row sharding; the batch-0 A2A and its half of
    the w2 projection hide under batch-1 compute,
  - each core owns 256 rows of each batch; host reassembles.

Matmuls run in float32r (TF32-like, ~1e-4 relative error, full PE rate at
free-dim >= 256).
"""

import math

import numpy as np

import concourse.bass as bass
import concourse.mybir as mybir
import concourse.tile as tile
from concourse import bacc
from concourse.bass_utils import run_bass_kernel_spmd

B, S, E, H = 2, 2048, 1024, 16
HD = E // H  # 64
BASE = 10000.0
N_CORES = 8
HPC = H // N_CORES       # heads per core = 2
R = B * S                # 4096 flattened rows
RT = 512                 # rows per r-tile
NEC = E // 128           # 8 e-chunks of 128
QT = 512                 # q columns per q-tile
N_QT = S // QT           # 4 q-tiles per batch
KC = 128                 # k rows per k-chunk
N_KC = S // KC           # 16 k-chunks per batch
RPB = S // N_CORES       # rows per core per batch = 256

F32 = mybir.dt.float32
F32R = mybir.dt.float32r
EXPF = mybir.ActivationFunctionType.Exp

_COMPILED = {}


def _build_nc():
    nc = bacc.Bacc("TRN2", target_bir_lowering=False, debug=False,
                   num_devices=N_CORES)

    xT = nc.dram_tensor("xT", [E, R], F32, kind="ExternalInput").ap()
    wqT = nc.dram_tensor("wqT", [E, 128], F32, kind="ExternalInput").ap()
    wkT = nc.dram_tensor("wkT", [E, 128], F32, kind="ExternalInput").ap()
    wvT = nc.dram_tensor("wvT", [E, 128], F32, kind="ExternalInput").ap()
    w2T = nc.dram_tensor("w2T", [E, E], F32, kind="ExternalInput").ap()
    cosT = nc.dram_tensor("cosT", [128, S], F32, kind="ExternalInput").ap()
    sinT = nc.dram_tensor("sinT", [128, S], F32, kind="ExternalInput").ap()
    p2T = nc.dram_tensor("p2T", [128, 128], F32, kind="ExternalInput").ap()
    out = nc.dram_tensor("out", [2 * RPB, E], F32, kind="ExternalOutput").ap()

    with tile.TileContext(nc) as tc:
        _emit(tc, nc, xT, wqT, wkT, wvT, w2T, cosT, sinT, p2T, out)
    nc.compile()
    return nc


def _emit(tc, nc, xT, wqT, wkT, wvT, w2T, cosT, sinT, p2T, out):
    import contextlib
    ctx = contextlib.ExitStack()
    consts = ctx.enter_context(tc.tile_pool(name="consts", bufs=1))
    xtp = ctx.enter_context(tc.tile_pool(name="xtp", bufs=2))
    qkp = ctx.enter_context(tc.tile_pool(name="qkp", bufs=1))
    rawp = ctx.enter_context(tc.tile_pool(name="rawp", bufs=2))
    tmpp = ctx.enter_context(tc.tile_pool(name="tmpp", bufs=2))
    vp = ctx.enter_context(tc.tile_pool(name="vp", bufs=1))
    pp = ctx.enter_context(tc.tile_pool(name="pp", bufs=6))
    smallp = ctx.enter_context(tc.tile_pool(name="smallp", bufs=2))
    dramp = ctx.enter_context(tc.tile_pool(name="dramp", bufs=1, space="DRAM"))
    # PSUM budget (8 banks): qkv-shared 2 + sps 2 x 2 + av 2 = 8
    ps_qkv = ctx.enter_context(tc.tile_pool(name="ps_qkv", bufs=2, space="PSUM"))
    ps_sps = ctx.enter_context(tc.tile_pool(name="ps_sps", bufs=2, space="PSUM"))
    ps_av = ctx.enter_context(tc.tile_pool(name="ps_av", bufs=2, space="PSUM"))

    # ---- batched constant loads (single DMA each) ----
    wq_all = consts.tile([128, NEC, 128], F32R, tag="wq", name="wq_all")
    nc.gpsimd.dma_start(out=wq_all[:], in_=wqT.rearrange("(c p) f -> p c f", p=128))
    wk_all = consts.tile([128, NEC, 128], F32R, tag="wk", name="wk_all")
    nc.gpsimd.dma_start(out=wk_all[:], in_=wkT.rearrange("(c p) f -> p c f", p=128))
    wv_all = consts.tile([128, NEC, 128], F32R, tag="wv", name="wv_all")
    nc.gpsimd.dma_start(out=wv_all[:], in_=wvT.rearrange("(c p) f -> p c f", p=128))
    p2_sb = consts.tile([128, 128], F32R, tag="p2", name="p2_sb")
    nc.gpsimd.dma_start(out=p2_sb[:], in_=p2T[:, :])

    cos_sb = consts.tile([128, S], F32, tag="cos", name="cos_sb")
    nc.sync.dma_start(out=cos_sb[:], in_=cosT[:, :])
    sin_sb = consts.tile([128, S], F32, tag="sin", name="sin_sb")
    nc.sync.dma_start(out=sin_sb[:], in_=sinT[:, :])
    ones_f32 = consts.tile([128, 64], F32, tag="ones32", name="ones_f32")
    nc.vector.memset(ones_f32[:], 1.0)

    # A2A buffers, one pair per batch: [8 chunks, 128 e-rows, 256 rows]
    send_d = [dramp.tile([N_CORES, 128, RPB], F32, name=f"send{b}")
              for b in range(B)]
    recv_d = [dramp.tile([N_CORES, 128, RPB], F32, name=f"recv{b}")
              for b in range(B)]

    qT_sb, kT_sb, v_sb = {}, {}, {}
    w2_sb = {}

    def emit_xt_load(rt):
        t = xtp.tile([128, NEC, RT], F32R, tag="xt", name=f"xt_{rt}")
        nc.gpsimd.dma_start(
            out=t[:],
            in_=xT.rearrange("(c p) r -> p c r", p=128)[:, :, rt * RT:(rt + 1) * RT])
        return t

    def qkv_chains(rt, xt):
        """Return a list of closures, each emitting one matmul chain (+ its
        epilogue) for r-tile rt. Callers dribble these between attention
        steps to keep the in-order PE stream dense but never monolithic."""
        b, st = rt // N_QT, (rt % N_QT) * RT

        if b not in qT_sb:
            qT_sb[b] = qkp.tile([128, S], F32R, tag=f"qT{b}", name=f"qT{b}")
            kT_sb[b] = qkp.tile([128, S], F32R, tag=f"kT{b}", name=f"kT{b}")

        def qk_chain(kind, w_all, dst):
            def emit():
                acc = ps_qkv.tile([128, RT], F32, tag="qkv",
                                  name=f"{kind}acc{rt}")
                for ec in range(NEC):
                    nc.tensor.matmul(acc[:], w_all[:, ec, :], xt[:, ec, :],
                                     start=(ec == 0), stop=(ec == NEC - 1))
                raw = rawp.tile([128, RT], F32R, tag="raw",
                                name=f"{kind}raw{rt}")
                nc.vector.tensor_copy(raw[:], acc[:])
                rot = ps_qkv.tile([128, RT], F32, tag="qkv",
                                  name=f"{kind}rot{rt}")
                nc.tensor.matmul(rot[:], p2_sb[:], raw[:], start=True, stop=True)
                t1 = tmpp.tile([128, RT], F32, tag="ropet", name=f"{kind}t1_{rt}")
                nc.vector.tensor_mul(t1[:], raw[:].bitcast(F32),
                                     cos_sb[:, st:st + RT])
                t2 = tmpp.tile([128, RT], F32, tag="ropet", name=f"{kind}t2_{rt}")
                nc.vector.tensor_mul(t2[:], rot[:], sin_sb[:, st:st + RT])
                nc.vector.tensor_add(dst[:, st:st + RT], t1[:], t2[:])
            return emit

        def v_chain(sub):
            def emit():
                vacc = ps_qkv.tile([128, 128], F32, tag="qkv",
                                   name=f"vacc{rt}_{sub}")
                for ec in range(NEC):
                    nc.tensor.matmul(vacc[:],
                                     xt[:, ec, sub * 128:(sub + 1) * 128],
                                     wv_all[:, ec, :],
                                     start=(ec == 0), stop=(ec == NEC - 1))
                kc = (rt % N_QT) * 4 + sub
                for h in range(HPC):
                    vt = vp.tile([128, 65], F32R, tag=f"v{b}{h}{kc}",
                                 name=f"v{b}{h}{kc}")
                    nc.vector.tensor_copy(vt[:, 0:64],
                                          vacc[:, h * 64:(h + 1) * 64])
                    nc.vector.tensor_copy(vt[:, 64:65], ones_f32[:, 0:1])
                    v_sb[(b, h, kc)] = vt
            return emit

        return [qk_chain("q", wq_all, qT_sb[b]),
                qk_chain("k", wk_all, kT_sb[b])] + \
               [v_chain(sub) for sub in range(4)]

    def proj_chains(b):
        """Output projection for my RPB rows of batch b, as dribble chains."""
        recv_sb = xtp.tile([128, NEC, RPB], F32R, tag="xt", name=f"recv{b}")
        nc.gpsimd.dma_start(out=recv_sb[:],
                            in_=recv_d[b].rearrange("c p r -> p c r"))
        chains = []
        for rblk in range(RPB // 128):
            for ft in range(2):
                def emit(rblk=rblk, ft=ft):
                    # qkv psum tag: free during attention (projection is done)
                    ops = ps_qkv.tile([128, 512], F32, tag="qkv",
                                      name=f"ops{b}_{rblk}_{ft}")
                    for ec in range(NEC):
                        nc.tensor.matmul(
                            ops[:],
                            recv_sb[:, ec, rblk * 128:(rblk + 1) * 128],
                            w2_sb[0][:, ec, ft * 512:(ft + 1) * 512],
                            start=(ec == 0), stop=(ec == NEC - 1))
                    ot = tmpp.tile([128, 512], F32, tag="ropet",
                                   name=f"ot{b}_{rblk}_{ft}")
                    nc.vector.tensor_copy(ot[:], ops[:])
                    nc.sync.dma_start(
                        out=out[b * RPB + rblk * 128:b * RPB + (rblk + 1) * 128,
                                ft * 512:(ft + 1) * 512],
                        in_=ot[:])
                chains.append(emit)
        return chains

    def emit_attention_qt(b, qt, dribble):
        """Both heads for one q-tile: packed scores, combined exp, rolling
        attn@v. Pops one chain off `dribble` every other k-chunk."""
        scale = 1.0 / math.sqrt(HD)
        avs = [ps_av.tile([65, QT], F32, tag="av", name=f"av{b}{h}{qt}")
               for h in range(HPC)]
        LAG = 3
        pts = {}
        for kc in range(N_KC + LAG):
            if kc < N_KC:
                sps = ps_sps.tile([128, 2 * QT], F32, tag="sps",
                                  name=f"s{b}{qt}_{kc}")
                for h in range(HPC):
                    hof = h * 64
                    nc.tensor.matmul(
                        sps[:, h * QT:(h + 1) * QT],
                        kT_sb[b][hof:hof + 64, kc * KC:(kc + 1) * KC],
                        qT_sb[b][hof:hof + 64, qt * QT:(qt + 1) * QT],
                        start=True, stop=True)
                pt = pp.tile([128, 2 * QT], F32R, tag="p", name=f"p{b}{qt}_{kc}")
                nc.scalar.activation(pt[:], sps[:], EXPF, scale=scale)
                pts[kc] = pt
            if kc >= LAG:
                j = kc - LAG
                for h in range(HPC):
                    nc.tensor.matmul(avs[h][:], v_sb[(b, h, j)][:],
                                     pts[j][:, h * QT:(h + 1) * QT],
                                     start=(j == 0), stop=(j == N_KC - 1))
                del pts[j]
            if kc % 2 == 1 and dribble:
                dribble.pop(0)()
        return avs

    def emit_divide(b, qt, avs):
        """Divide by the softmax denominator (row 64 of av) and stage into
        the A2A send buffer. PE-free: broadcast via a DRAM bounce DMA."""
        for h in range(HPC):
            av = avs[h]
            rcp = smallp.tile([1, QT], F32, tag="rcp", name=f"rcp{b}{h}{qt}")
            nc.vector.reciprocal(rcp[:], av[64:65, :])
            rcp_d = dramp.tile([1, QT], F32, tag="rcpd", bufs=4,
                               name=f"rcpd{b}{h}{qt}")
            nc.sync.dma_start(out=rcp_d[:], in_=rcp[:])
            bcs = smallp.tile([64, QT], F32, tag="bcs", name=f"bcs{b}{h}{qt}")
            bcast = bass.AP(tensor=rcp_d.tensor, offset=rcp_d.offset,
                            ap=[[0, 64]] + list(rcp_d.ap[1:]))
            nc.gpsimd.dma_start(out=bcs[:], in_=bcast)
            odiv = smallp.tile([64, QT], F32, tag="odiv", name=f"odiv{b}{h}{qt}")
            nc.vector.tensor_mul(odiv[:], av[0:64, :], bcs[:])
            nc.sync.dma_start(out=send_d[b][2 * qt, h * 64:(h + 1) * 64, :],
                              in_=odiv[:, 0:RPB])
            nc.sync.dma_start(out=send_d[b][2 * qt + 1, h * 64:(h + 1) * 64, :],
                              in_=odiv[:, RPB:2 * RPB])

    def emit_a2a(b):
        nc.gpsimd.collective_compute(
            "AllToAll", mybir.AluOpType.bypass,
            replica_groups=[list(range(N_CORES))],
            ins=[send_d[b].opt()], outs=[recv_d[b].opt()])

    # ---------------- emission ----------------
    for rt in range(N_QT):             # batch-0 projection: pure PE stretch
        xt = emit_xt_load(rt)
        for chain in qkv_chains(rt, xt):
            chain()

    # batch-0 attention with batch-1 qkv dribbled in
    dribble = []
    for qt in range(N_QT):
        rt = N_QT + qt
        xt = emit_xt_load(rt)
        dribble.extend(qkv_chains(rt, xt))
        avs = emit_attention_qt(0, qt, dribble)
        emit_divide(0, qt, avs)
    # w2 (single 4 MB DMA; needed from the first out-projection onward)
    w2_sb[0] = consts.tile([128, NEC, E], F32R, tag="w2", name="w2_all")
    nc.gpsimd.dma_start(out=w2_sb[0][:],
                        in_=w2T.rearrange("(c p) f -> p c f", p=128))
    emit_a2a(0)                        # in flight under batch-1 attention

    dribble = list(dribble)            # leftovers, if any
    for qt in range(N_QT):
        if qt == 2:
            dribble.extend(proj_chains(0))   # A2A#0 done by now
        avs = emit_attention_qt(1, qt, dribble)
        emit_divide(1, qt, avs)
    for chain in dribble:
        chain()
    emit_a2a(1)
    for chain in proj_chains(1):
        chain()
    ctx.close()


def _host_prep(x, w1, w2):
    x = np.ascontiguousarray(np.asarray(x, dtype=np.float32))
    w1 = np.ascontiguousarray(np.asarray(w1, dtype=np.float32))
    w2 = np.ascontiguousarray(np.asarray(w2, dtype=np.float32))

    xT = np.ascontiguousarray(x.reshape(R, E).T)          # [E, R]
    w2T = np.ascontiguousarray(w2.T)                      # [E, E]

    theta = 1.0 / (BASE ** (np.arange(0, HD, 2, dtype=np.float32) / HD))
    enc = np.arange(S, dtype=np.float32)[:, None] * theta[None, :]
    enc = np.repeat(enc, 2, axis=-1)                      # [s, 64]
    cos1 = np.cos(enc).T.astype(np.float32)               # [64, S]
    sin1 = np.sin(enc).T.astype(np.float32)
    cosT = np.ascontiguousarray(np.concatenate([cos1, cos1], axis=0))
    sinT = np.ascontiguousarray(np.concatenate([sin1, sin1], axis=0))

    m64 = np.zeros((HD, HD), dtype=np.float32)
    for i in range(HD // 2):
        m64[2 * i, 2 * i + 1] = -1.0
        m64[2 * i + 1, 2 * i] = 1.0
    m128 = np.zeros((128, 128), dtype=np.float32)
    m128[:64, :64] = m64
    m128[64:, 64:] = m64
    p2T = np.ascontiguousarray(m128.T)

    in_maps = []
    for c in range(N_CORES):
        hA, hB = HPC * c, HPC * c + 1
        def rows(base):
            return np.concatenate(
                [w1[base + hA * HD: base + (hA + 1) * HD, :],
                 w1[base + hB * HD: base + (hB + 1) * HD, :]], axis=0)
        in_maps.append({
            "xT": xT,
            "wqT": np.ascontiguousarray(rows(0).T),
            "wkT": np.ascontiguousarray(rows(E).T),
            "wvT": np.ascontiguousarray(rows(2 * E).T),
            "w2T": w2T,
            "cosT": cosT,
            "sinT": sinT,
            "p2T": p2T,
        })
    return in_maps


def kernel(x, w1, w2, _trace=False):
    if "nc" not in _COMPILED:
        _COMPILED["nc"] = _build_nc()
    nc = _COMPILED["nc"]
    in_maps = _host_prep(x, w1, w2)
    res = run_bass_kernel_spmd(nc, in_maps, core_ids=list(range(N_CORES)),
                               trace=_trace)
    _COMPILED["last_result"] = res
    # core c returns [512, E]: rows 0..255 = batch0 s in [256c, 256c+256),
    # rows 256..511 = batch1 same s range.
    full = np.empty((B, S, E), dtype=np.float32)
    for c in range(N_CORES):
        blk = res.results[c]["out"]
        full[0, c * RPB:(c + 1) * RPB] = blk[0:RPB]
        full[1, c * RPB:(c + 1) * RPB] = blk[RPB:2 * RPB]
    return full
